# revision 3
# baseline (speedup 1.0000x reference)
"""Trainium2 Bass kernel for nn_AutoSparseLinear: out = sparse @ weight + b.

Shapes (hardcoded): sparse [4096, 4096] f32, weight [4096, 4096] f32,
b [4096] f32 -> out [4096, 4096] f32.

Strategy: 2D shard across 8 cores as 4 batch-shards x 2 column-shards.
Core c = 4*cs + br computes out[br*1024:(br+1)*1024, cs*2048:(cs+1)*2048].
This cuts per-core HBM traffic to x 8.4 MiB (SBUF-resident) + W 16.8 MiB
(streamed once) + out 4.2 MiB, vs ~41.5 MiB for pure batch sharding -
on this part DMA time adds to PE time, so fewer streamed bytes wins.

Per core: out_shard^T = Wshard^T @ xshard^T on the PE with W tiles
stationary and x^T the moving operand. fp16 operands run the PE at
1 cycle/row: 16 n-tiles x 2 m-halves x 32 k-tiles = 1024 matmuls of 512
moving columns = 218.1 us/core PE floor, which the steady state hits
exactly (TimelineSim marginal per-rep = 218112 ns). fp8 cannot pass the
2e-2 gate (e4m3 x-side error alone is 2.2e-2; DoubleRow pairs buy speed
or 2-level precision, never both), so 16-bit is the compute floor.

Ramp: no PSUM group can finish before the whole x shard lands, so rep 0
runs a progressive-K wave - 8 PSUM groups (n-tiles 0..3 x 2 m-halves)
stay live and accumulate k-chunk by k-chunk as the x chunks (sizes
1,1,2,4,8,8,8 k-tiles) arrive; the matching W k-ranges for all 4 wave
tiles stream as one strided DMA per chunk. Each 1 MiB x chunk enables
6.8 us of PE work, exceeding its arrival time, so the PE runs gap-free
from ~4 us. Remaining n-tiles stream whole 1 MiB W tiles on the scalar
ring (x/bias/stores use the sync ring). Bias is added during the
PSUM->SBUF eviction on the vector engine; both m-halves of an n-tile
coalesce into a single bf16 store (16 stores, fewer serialized HWDGE
descriptor slots). Total absmax rel err ~2.3e-3 (budget 2e-2).

Host side only reshapes/transposes/casts for layout and reassembles the
output block grid.
"""

import numpy as np

import concourse.bass as bass
import concourse.mybir as mybir
import concourse.tile as tile
from concourse import bacc
from concourse.bass_utils import run_bass_kernel_spmd

P = 128
B = 4096
NCORES = 8
BR = 4            # batch shards
CS = 2            # column shards
MB = B // BR      # 1024 batch rows per core
MH = 512          # moving columns per matmul
NMH = MB // MH    # 2 m-halves
K = 4096
N = 4096
NC_ = N // CS     # 2048 out features per core
KT = K // P       # 32
NT = NC_ // P     # 16 n-tiles per core
XCHUNKS = [1, 1, 2, 4, 8, 8, 8]  # x chunk sizes in k-tiles
XOFF = [0]
for _s in XCHUNKS:
    XOFF.append(XOFF[-1] + _s)
assert XOFF[-1] == KT
WAVE_NT = 4       # n-tiles in the ramp wave

MM_DT = mybir.dt.float16
NP_DT = np.float16
OUT_DT = mybir.dt.bfloat16

_CACHE = {}


def build_nc(repeat=1):
    nc = bacc.Bacc("TRN2", target_bir_lowering=False, debug=False)

    xT = nc.dram_tensor("xT", [P, KT * MB], MM_DT, kind="ExternalInput").ap()
    w = nc.dram_tensor("w", [NT, P, KT * P], MM_DT, kind="ExternalInput").ap()
    bias = nc.dram_tensor("bias", [P, NT], mybir.dt.float32,
                          kind="ExternalInput").ap()
    outT = nc.dram_tensor("outT", [NT, P, MB], OUT_DT,
                          kind="ExternalOutput").ap()

    with tile.TileContext(nc) as tc:
        with (
            tc.tile_pool(name="xpool", bufs=1) as xpool,
            tc.tile_pool(name="wpool", bufs=5) as wpool,
            tc.tile_pool(name="wavepool", bufs=1) as wavepool,
            tc.tile_pool(name="opool", bufs=4) as opool,
            tc.tile_pool(name="bpool", bufs=1) as bpool,
            tc.tile_pool(name="pspool", bufs=8, space="PSUM") as pspool,
        ):
            xch = [xpool.tile([P, sz * MB], MM_DT, name=f"xc{c}",
                               tag=f"xc{c}")
                   for c, sz in enumerate(XCHUNKS)]

            def load_xchunk(c):
                nc.sync.dma_start(xch[c][:],
                                  xT[:, XOFF[c] * MB:XOFF[c + 1] * MB])

            bt = bpool.tile([P, NT], mybir.dt.float32)

            def xslice(kt, mh):
                c = 0
                while XOFF[c + 1] <= kt:
                    c += 1
                j = kt - XOFF[c]
                return xch[c][:, (j * NMH + mh) * MH:(j * NMH + mh + 1) * MH]

            def steady_tile(r, nt, wt):
                ot = opool.tile([P, MB], OUT_DT, name=f"ot{r}_{nt}", tag="ot")
                for mh in range(NMH):
                    ps = pspool.tile([P, MH], mybir.dt.float32,
                                     name=f"ps{r}_{nt}_{mh}", tag="ps")
                    for kt in range(KT):
                        nc.tensor.matmul(
                            ps[:], wt[:, kt * P:(kt + 1) * P], xslice(kt, mh),
                            start=(kt == 0), stop=(kt == KT - 1),
                        )
                    evict(nt, mh, ps, ot)
                nc.sync.dma_start(outT[nt], ot[:])

            def evict(nt, mh, ps, ot):
                nc.vector.tensor_scalar_add(ot[:, mh * MH:(mh + 1) * MH],
                                            ps[:], bt[:, nt:nt + 1])

            for r in range(repeat):
                if r == 0:
                    # ramp wave over n-tiles 0..WAVE_NT-1
                    wavet = wavepool.tile([P, WAVE_NT * KT * P], MM_DT,
                                       name="wt0_wave", tag="wtwave")
                    wav3d = wavet[:].rearrange("p (n m) -> p n m", n=WAVE_NT)
                    wsrc = w.rearrange("n p m -> p n m")
                    # interleave issue: xc0, W-chunk0, xc1, W-chunk1, ...
                    # so the first matmul's operands get the first DMA slots
                    for c in range(len(XCHUNKS)):
                        load_xchunk(c)
                        lo, hi = XOFF[c] * P, XOFF[c + 1] * P
                        nc.scalar.dma_start(wav3d[:, :, lo:hi],
                                            wsrc[:, 0:WAVE_NT, lo:hi])
                    nc.sync.dma_start(bt[:], bias[:])
                    wts = [wavet[:, nt * KT * P:(nt + 1) * KT * P]
                           for nt in range(WAVE_NT)]
                    groups = [(nt, mh) for nt in range(WAVE_NT)
                              for mh in range(NMH)]
                    pss = {}
                    for nt, mh in groups:
                        pss[(nt, mh)] = pspool.tile(
                            [P, MH], mybir.dt.float32,
                            name=f"ps0_{nt}_{mh}", tag="ps")
                    for c in range(len(XCHUNKS)):
                        for nt, mh in groups:
                            for j in range(XCHUNKS[c]):
                                kt = XOFF[c] + j
                                nc.tensor.matmul(
                                    pss[(nt, mh)][:],
                                    wts[nt][:, kt * P:(kt + 1) * P],
                                    xslice(kt, mh),
                                    start=(kt == 0), stop=(kt == KT - 1),
                                )
                    for nt in range(WAVE_NT):
                        ot = opool.tile([P, MB], OUT_DT, name=f"ot0_{nt}",
                                        tag="ot")
                        for mh in range(NMH):
                            evict(nt, mh, pss[(nt, mh)], ot)
                        nc.sync.dma_start(outT[nt], ot[:])
                    rest = range(WAVE_NT, NT)
                else:
                    rest = range(NT)
                for nt in rest:
                    wt = wpool.tile([P, KT * P], MM_DT, name=f"wt{r}_{nt}",
                                    tag="wt")
                    nc.scalar.dma_start(wt[:], w[nt])
                    steady_tile(r, nt, wt)

    nc.compile()
    return nc


def get_nc():
    if "nc" not in _CACHE:
        _CACHE["nc"] = build_nc()
    return _CACHE["nc"]


def shard_inputs(sparse, weight, b):
    sparse = np.asarray(sparse)
    weight = np.asarray(weight).astype(NP_DT)
    b = np.ascontiguousarray(np.asarray(b), dtype=np.float32)

    wshards = []
    bshards = []
    for cs in range(CS):
        ws = weight[:, cs * NC_:(cs + 1) * NC_]
        wb = np.ascontiguousarray(
            ws.reshape(KT, P, NT, P).transpose(2, 1, 0, 3)
            .reshape(NT, P, KT * P)
        )
        wshards.append(wb)
        bs = b[cs * NC_:(cs + 1) * NC_]
        bshards.append(np.ascontiguousarray(bs.reshape(NT, P).T))

    in_maps = []
    for c in range(NCORES):
        cs, br = divmod(c, BR)
        xs = sparse[br * MB:(br + 1) * MB, :].astype(NP_DT)
        xb = np.ascontiguousarray(
            xs.reshape(NMH, MH, KT, P).transpose(3, 2, 0, 1)
            .reshape(P, KT * MB)
        )
        in_maps.append({"xT": xb, "w": wshards[cs], "bias": bshards[cs]})
    return in_maps


def unshard_output(results):
    out = np.empty((B, N), dtype=np.float32)
    for c in range(NCORES):
        cs, br = divmod(c, BR)
        oT = results[c]["outT"].astype(np.float32)
        out[br * MB:(br + 1) * MB, cs * NC_:(cs + 1) * NC_] = \
            oT.reshape(NC_, MB).T
    return np.ascontiguousarray(out)


def kernel(sparse, weight, b, **run_kwargs):
    nc = get_nc()
    in_maps = shard_inputs(sparse, weight, b)
    res = run_bass_kernel_spmd(nc, in_maps, core_ids=list(range(NCORES)),
                               **run_kwargs)
    out = unshard_output(res.results)
    if run_kwargs:
        _CACHE["last_result"] = res
    return out


# revision 4
# speedup vs baseline: 1.0348x; 1.0348x over previous
"""Trainium2 Bass kernel for nn_AutoSparseLinear: out = sparse @ weight + b.

Shapes (hardcoded): sparse [4096, 4096] f32, weight [4096, 4096] f32,
b [4096] f32 -> out [4096, 4096] f32.

Strategy: 2D shard across 8 cores as 4 batch-shards x 2 column-shards.
Core c = 4*cs + br computes out[br*1024:(br+1)*1024, cs*2048:(cs+1)*2048].
The column split makes the per-core W shard 16.8 MiB = 128 KiB/partition,
small enough to keep FULLY SBUF-RESIDENT next to the 64 KiB/partition x
shard (~198 of ~212 KiB/partition): W streams from HBM exactly once and
all later use is SBUF reads. Total per-core HBM traffic is x 8.4 MiB +
W 16.8 MiB + out 4.2 MiB vs ~41.5 MiB for pure batch sharding - on this
part DMA time adds to PE time, so streamed bytes are the scarce resource.

Per core: out_shard^T = Wshard^T @ xshard^T on the PE with W tiles
stationary and x^T the moving operand. fp16 operands run the PE at
1 cycle/row: 16 n-tiles x 2 m-halves x 32 k-tiles = 1024 matmuls of 512
moving columns = 218.1 us/core PE floor, which the steady state hits
exactly (TimelineSim marginal per-rep = 218112 ns). fp8 cannot pass the
2e-2 gate (e4m3 x-side error alone is 2.2e-2; DoubleRow pairs buy speed
or 2-level precision, never both), so 16-bit is the compute floor.

Ramp: no PSUM group can finish before the whole x shard lands, so the
first pass runs a progressive-K wave - 8 PSUM groups (n-tiles 0..3 x 2
m-halves) stay live and accumulate k-chunk by k-chunk as the x chunks
(sizes 1,1,2,4,8,8,8 k-tiles) arrive; the matching W k-ranges for all 4
wave tiles fill their resident regions via one strided DMA per chunk.
Each 1 MiB x chunk enables 6.8 us of PE work, exceeding its arrival
time, so the PE runs gap-free from ~4 us. W streams on the scalar ring;
x, bias and stores use the sync ring. Bias is added during the
PSUM->SBUF eviction on the vector engine; both m-halves of an n-tile
coalesce into a single bf16 store. Absmax rel err ~2.3e-3 (budget 2e-2).

Host side only reshapes/transposes/casts for layout and reassembles the
output block grid.
"""

import numpy as np

import concourse.bass as bass
import concourse.mybir as mybir
import concourse.tile as tile
from concourse import bacc
from concourse.bass_utils import run_bass_kernel_spmd

P = 128
B = 4096
NCORES = 8
BR = 4            # batch shards
CS = 2            # column shards
MB = B // BR      # 1024 batch rows per core
MH = 512          # moving columns per matmul
NMH = MB // MH    # 2 m-halves
K = 4096
N = 4096
NC_ = N // CS     # 2048 out features per core
KT = K // P       # 32
NT = NC_ // P     # 16 n-tiles per core
XCHUNKS = [1, 1, 2, 4, 8, 8, 8]  # x chunk sizes in k-tiles
XOFF = [0]
for _s in XCHUNKS:
    XOFF.append(XOFF[-1] + _s)
assert XOFF[-1] == KT
WAVE_NT = 4       # n-tiles in the ramp wave

MM_DT = mybir.dt.float16
NP_DT = np.float16
OUT_DT = mybir.dt.bfloat16

_CACHE = {}


def build_nc(repeat=1):
    nc = bacc.Bacc("TRN2", target_bir_lowering=False, debug=False)

    xT = nc.dram_tensor("xT", [P, KT * MB], MM_DT, kind="ExternalInput").ap()
    w = nc.dram_tensor("w", [NT, P, KT * P], MM_DT, kind="ExternalInput").ap()
    bias = nc.dram_tensor("bias", [P, NT], mybir.dt.float32,
                          kind="ExternalInput").ap()
    outT = nc.dram_tensor("outT", [NT, P, MB], OUT_DT,
                          kind="ExternalOutput").ap()

    with tile.TileContext(nc) as tc:
        with (
            tc.tile_pool(name="xpool", bufs=1) as xpool,
            tc.tile_pool(name="wpool", bufs=1) as wpool,
            tc.tile_pool(name="opool", bufs=3) as opool,
            tc.tile_pool(name="bpool", bufs=1) as bpool,
            tc.tile_pool(name="pspool", bufs=8, space="PSUM") as pspool,
        ):
            xch = [xpool.tile([P, sz * MB], MM_DT, name=f"xc{c}",
                               tag=f"xc{c}")
                   for c, sz in enumerate(XCHUNKS)]

            def load_xchunk(c):
                nc.sync.dma_start(xch[c][:],
                                  xT[:, XOFF[c] * MB:XOFF[c + 1] * MB])

            bt = bpool.tile([P, NT], mybir.dt.float32)

            def xslice(kt, mh):
                c = 0
                while XOFF[c + 1] <= kt:
                    c += 1
                j = kt - XOFF[c]
                return xch[c][:, (j * NMH + mh) * MH:(j * NMH + mh + 1) * MH]

            def steady_tile(r, nt, wt):
                ot = opool.tile([P, MB], OUT_DT, name=f"ot{r}_{nt}", tag="ot")
                for mh in range(NMH):
                    ps = pspool.tile([P, MH], mybir.dt.float32,
                                     name=f"ps{r}_{nt}_{mh}", tag="ps")
                    for kt in range(KT):
                        nc.tensor.matmul(
                            ps[:], wt[:, kt * P:(kt + 1) * P], xslice(kt, mh),
                            start=(kt == 0), stop=(kt == KT - 1),
                        )
                    evict(nt, mh, ps, ot)
                nc.sync.dma_start(outT[nt], ot[:])

            def evict(nt, mh, ps, ot):
                nc.vector.tensor_scalar_add(ot[:, mh * MH:(mh + 1) * MH],
                                            ps[:], bt[:, nt:nt + 1])

            for r in range(repeat):
                if r == 0:
                    # W stays fully SBUF-resident: one 128 KiB/partition
                    # tile holding the whole 16.8 MiB shard, streamed once.
                    wres = wpool.tile([P, NT * KT * P], MM_DT,
                                      name="wres", tag="wres")
                    wr3d = wres[:].rearrange("p (n m) -> p n m", n=NT)
                    wsrc = w.rearrange("n p m -> p n m")
                    # ramp wave over n-tiles 0..WAVE_NT-1: per x-chunk, one
                    # strided DMA fills the matching k-range of all wave tiles
                    for c in range(len(XCHUNKS)):
                        load_xchunk(c)
                        lo, hi = XOFF[c] * P, XOFF[c + 1] * P
                        nc.scalar.dma_start(wr3d[:, 0:WAVE_NT, lo:hi],
                                            wsrc[:, 0:WAVE_NT, lo:hi])
                    nc.sync.dma_start(bt[:], bias[:])
                    wts = [wres[:, nt * KT * P:(nt + 1) * KT * P]
                           for nt in range(WAVE_NT)]
                    groups = [(nt, mh) for nt in range(WAVE_NT)
                              for mh in range(NMH)]
                    pss = {}
                    for nt, mh in groups:
                        pss[(nt, mh)] = pspool.tile(
                            [P, MH], mybir.dt.float32,
                            name=f"ps0_{nt}_{mh}", tag="ps")
                    for c in range(len(XCHUNKS)):
                        for nt, mh in groups:
                            for j in range(XCHUNKS[c]):
                                kt = XOFF[c] + j
                                nc.tensor.matmul(
                                    pss[(nt, mh)][:],
                                    wts[nt][:, kt * P:(kt + 1) * P],
                                    xslice(kt, mh),
                                    start=(kt == 0), stop=(kt == KT - 1),
                                )
                    for nt in range(WAVE_NT):
                        ot = opool.tile([P, MB], OUT_DT, name=f"ot0_{nt}",
                                        tag="ot")
                        for mh in range(NMH):
                            evict(nt, mh, pss[(nt, mh)], ot)
                        nc.sync.dma_start(outT[nt], ot[:])
                    rest = range(WAVE_NT, NT)
                else:
                    rest = range(NT)
                for nt in rest:
                    wt = wres[:, nt * KT * P:(nt + 1) * KT * P]
                    if r == 0 and nt >= WAVE_NT:
                        nc.scalar.dma_start(wt, w[nt])
                    steady_tile(r, nt, wt)

    nc.compile()
    return nc


def get_nc():
    if "nc" not in _CACHE:
        _CACHE["nc"] = build_nc()
    return _CACHE["nc"]


def shard_inputs(sparse, weight, b):
    sparse = np.asarray(sparse)
    weight = np.asarray(weight).astype(NP_DT)
    b = np.ascontiguousarray(np.asarray(b), dtype=np.float32)

    wshards = []
    bshards = []
    for cs in range(CS):
        ws = weight[:, cs * NC_:(cs + 1) * NC_]
        wb = np.ascontiguousarray(
            ws.reshape(KT, P, NT, P).transpose(2, 1, 0, 3)
            .reshape(NT, P, KT * P)
        )
        wshards.append(wb)
        bs = b[cs * NC_:(cs + 1) * NC_]
        bshards.append(np.ascontiguousarray(bs.reshape(NT, P).T))

    in_maps = []
    for c in range(NCORES):
        cs, br = divmod(c, BR)
        xs = sparse[br * MB:(br + 1) * MB, :].astype(NP_DT)
        xb = np.ascontiguousarray(
            xs.reshape(NMH, MH, KT, P).transpose(3, 2, 0, 1)
            .reshape(P, KT * MB)
        )
        in_maps.append({"xT": xb, "w": wshards[cs], "bias": bshards[cs]})
    return in_maps


def unshard_output(results):
    out = np.empty((B, N), dtype=np.float32)
    for c in range(NCORES):
        cs, br = divmod(c, BR)
        oT = results[c]["outT"].astype(np.float32)
        out[br * MB:(br + 1) * MB, cs * NC_:(cs + 1) * NC_] = \
            oT.reshape(NC_, MB).T
    return np.ascontiguousarray(out)


def kernel(sparse, weight, b, **run_kwargs):
    nc = get_nc()
    in_maps = shard_inputs(sparse, weight, b)
    res = run_bass_kernel_spmd(nc, in_maps, core_ids=list(range(NCORES)),
                               **run_kwargs)
    out = unshard_output(res.results)
    if run_kwargs:
        _CACHE["last_result"] = res
    return out


# revision 5
# speedup vs baseline: 1.2622x; 1.2198x over previous
"""Trainium2 Bass kernel for nn_AutoSparseLinear: out = sparse @ weight + b.

Shapes (hardcoded): sparse [4096, 4096] f32, weight [4096, 4096] f32,
b [4096] f32 -> out [4096, 4096] f32.

Strategy: 2D shard across 8 cores as 4 batch-shards x 2 column-shards;
core c = 4*cs + br computes out[br*1024:(br+1)*1024, cs*2048:(cs+1)*2048].
The column split makes the per-core W shard 16.8 MiB = 128 KiB/partition,
kept FULLY SBUF-RESIDENT next to the 64 KiB/partition x shard: W streams
from HBM exactly once. fp16 runs the PE at 1 cycle/row (fp8 fails the
2e-2 gate); 1024 matmuls of 512 moving columns = 218.1 us/core PE floor,
hit exactly in steady state. Ramp: PE-clock warmup matmuls on a memset
scratch burn the initial DMA wait so real matmuls start HAM-warm at
2.4 GHz, then a progressive-K wave (8 live PSUM groups over n-tiles 0-3,
x chunks of 1,1,2,4,8,8,8 k-tiles, wave W k-ranges as one strided DMA
per chunk) keeps the PE gap-free while x lands. W uses the scalar ring;
x/bias/stores the sync ring; bias is added during PSUM->SBUF eviction on
the vector engine; both m-halves of an n-tile coalesce into one bf16
store. Absmax rel err ~2.3e-3 (budget 2e-2).

Ramp problem in v3: no PSUM group can complete until the whole 8.4 MiB x
shard has landed, so the PE idles ~12us and runs cold early. Fix: during
rep 0, keep 8 PSUM groups live (n-tiles 0..3 x 2 m-halves) and accumulate
them k-chunk by k-chunk as the 8 x-chunks arrive; W tiles 0..3 stream in
matching k-chunk-major order on the scalar ring. Each 1 MiB x chunk
enables 8 groups x 4 k-tiles x 213 ns = 6.8 us of PE work, which exceeds
the chunk arrival time even at half DMA bandwidth, so the PE stays busy
from ~1.5 us onward. n-tiles 4..15 (and all repeats) run the plain loop.
"""

import numpy as np

import concourse.bass as bass
import concourse.mybir as mybir
import concourse.tile as tile
from concourse import bacc
from concourse.bass_utils import run_bass_kernel_spmd

P = 128
B = 4096
NCORES = 8
BR = 4            # batch shards
CS = 2            # column shards
MB = B // BR      # 1024 batch rows per core
MH = 512          # moving columns per matmul
NMH = MB // MH    # 2 m-halves
K = 4096
N = 4096
NC_ = N // CS     # 2048 out features per core
KT = K // P       # 32
NT = NC_ // P     # 16 n-tiles per core
XCHUNKS = [1, 1, 2, 4, 8, 8, 8]  # x chunk sizes in k-tiles
XOFF = [0]
for _s in XCHUNKS:
    XOFF.append(XOFF[-1] + _s)
assert XOFF[-1] == KT
WAVE_NT = 4       # n-tiles in the ramp wave

MM_DT = mybir.dt.float16
NP_DT = np.float16
OUT_DT = mybir.dt.bfloat16

_CACHE = {}


def build_nc(repeat=1):
    nc = bacc.Bacc("TRN2", target_bir_lowering=False, debug=False)

    xT = nc.dram_tensor("xT", [P, KT * MB], MM_DT, kind="ExternalInput").ap()
    w = nc.dram_tensor("w", [NT, P, KT * P], MM_DT, kind="ExternalInput").ap()
    bias = nc.dram_tensor("bias", [P, NT], mybir.dt.float32,
                          kind="ExternalInput").ap()
    outT = nc.dram_tensor("outT", [NT, P, MB], OUT_DT,
                          kind="ExternalOutput").ap()

    with tile.TileContext(nc) as tc:
        with (
            tc.tile_pool(name="xpool", bufs=1) as xpool,
            tc.tile_pool(name="wpool", bufs=1) as wpool,
            tc.tile_pool(name="opool", bufs=3) as opool,
            tc.tile_pool(name="bpool", bufs=1) as bpool,
            tc.tile_pool(name="pspool", bufs=8, space="PSUM") as pspool,
        ):
            xch = [xpool.tile([P, sz * MB], MM_DT, name=f"xc{c}",
                               tag=f"xc{c}")
                   for c, sz in enumerate(XCHUNKS)]

            def load_xchunk(c):
                nc.sync.dma_start(xch[c][:],
                                  xT[:, XOFF[c] * MB:XOFF[c + 1] * MB])

            bt = bpool.tile([P, NT], mybir.dt.float32)

            def xslice(kt, mh):
                c = 0
                while XOFF[c + 1] <= kt:
                    c += 1
                j = kt - XOFF[c]
                return xch[c][:, (j * NMH + mh) * MH:(j * NMH + mh + 1) * MH]

            def steady_tile(r, nt, wt):
                ot = opool.tile([P, MB], OUT_DT, name=f"ot{r}_{nt}", tag="ot")
                for mh in range(NMH):
                    ps = pspool.tile([P, MH], mybir.dt.float32,
                                     name=f"ps{r}_{nt}_{mh}", tag="ps")
                    for kt in range(KT):
                        nc.tensor.matmul(
                            ps[:], wt[:, kt * P:(kt + 1) * P], xslice(kt, mh),
                            start=(kt == 0), stop=(kt == KT - 1),
                        )
                    evict(nt, mh, ps, ot)
                nc.sync.dma_start(outT[nt], ot[:])

            def evict(nt, mh, ps, ot):
                nc.vector.tensor_scalar_add(ot[:, mh * MH:(mh + 1) * MH],
                                            ps[:], bt[:, nt:nt + 1])

            for r in range(repeat):
                if r == 0:
                    # PE clock warmup: the HAM gate keeps the PE at 1.2 GHz
                    # until ~3.4us of sustained activity. Burn the initial
                    # DMA wait with junk matmuls on a memset scratch tile so
                    # real matmuls start at 2.4 GHz. Writes land in a wave
                    # PSUM bank that the real accumulation later overwrites
                    # with start=True.
                    scr = bpool.tile([P, P + MH], MM_DT, name="wuscr",
                                     tag="wuscr")
                    nc.vector.memset(scr[:], 0.0)
                    # W stays fully SBUF-resident: one 128 KiB/partition
                    # tile holding the whole 16.8 MiB shard, streamed once.
                    wres = wpool.tile([P, NT * KT * P], MM_DT,
                                      name="wres", tag="wres")
                    wr3d = wres[:].rearrange("p (n m) -> p n m", n=NT)
                    wsrc = w.rearrange("n p m -> p n m")
                    # ramp wave over n-tiles 0..WAVE_NT-1: per x-chunk, one
                    # strided DMA fills the matching k-range of all wave tiles
                    for c in range(len(XCHUNKS)):
                        load_xchunk(c)
                        lo, hi = XOFF[c] * P, XOFF[c + 1] * P
                        nc.scalar.dma_start(wr3d[:, 0:WAVE_NT, lo:hi],
                                            wsrc[:, 0:WAVE_NT, lo:hi])
                    nc.sync.dma_start(bt[:], bias[:])
                    wts = [wres[:, nt * KT * P:(nt + 1) * KT * P]
                           for nt in range(WAVE_NT)]
                    groups = [(nt, mh) for nt in range(WAVE_NT)
                              for mh in range(NMH)]
                    pss = {}
                    for nt, mh in groups:
                        pss[(nt, mh)] = pspool.tile(
                            [P, MH], mybir.dt.float32,
                            name=f"ps0_{nt}_{mh}", tag="ps")
                    for wu in range(12):
                        nc.tensor.matmul(pss[groups[0]][:], scr[:, :P],
                                         scr[:, P:], start=True, stop=True)
                    for c in range(len(XCHUNKS)):
                        for nt, mh in groups:
                            for j in range(XCHUNKS[c]):
                                kt = XOFF[c] + j
                                nc.tensor.matmul(
                                    pss[(nt, mh)][:],
                                    wts[nt][:, kt * P:(kt + 1) * P],
                                    xslice(kt, mh),
                                    start=(kt == 0), stop=(kt == KT - 1),
                                )
                    for nt in range(WAVE_NT):
                        ot = opool.tile([P, MB], OUT_DT, name=f"ot0_{nt}",
                                        tag="ot")
                        for mh in range(NMH):
                            evict(nt, mh, pss[(nt, mh)], ot)
                        nc.sync.dma_start(outT[nt], ot[:])
                    rest = range(WAVE_NT, NT)
                else:
                    rest = range(NT)
                for nt in rest:
                    wt = wres[:, nt * KT * P:(nt + 1) * KT * P]
                    if r == 0 and nt >= WAVE_NT:
                        nc.scalar.dma_start(wt, w[nt])
                    steady_tile(r, nt, wt)

    nc.compile()
    return nc


def get_nc():
    if "nc" not in _CACHE:
        _CACHE["nc"] = build_nc()
    return _CACHE["nc"]


def shard_inputs(sparse, weight, b):
    sparse = np.asarray(sparse)
    weight = np.asarray(weight).astype(NP_DT)
    b = np.ascontiguousarray(np.asarray(b), dtype=np.float32)

    wshards = []
    bshards = []
    for cs in range(CS):
        ws = weight[:, cs * NC_:(cs + 1) * NC_]
        wb = np.ascontiguousarray(
            ws.reshape(KT, P, NT, P).transpose(2, 1, 0, 3)
            .reshape(NT, P, KT * P)
        )
        wshards.append(wb)
        bs = b[cs * NC_:(cs + 1) * NC_]
        bshards.append(np.ascontiguousarray(bs.reshape(NT, P).T))

    in_maps = []
    for c in range(NCORES):
        cs, br = divmod(c, BR)
        xs = sparse[br * MB:(br + 1) * MB, :].astype(NP_DT)
        xb = np.ascontiguousarray(
            xs.reshape(NMH, MH, KT, P).transpose(3, 2, 0, 1)
            .reshape(P, KT * MB)
        )
        in_maps.append({"xT": xb, "w": wshards[cs], "bias": bshards[cs]})
    return in_maps


def unshard_output(results):
    out = np.empty((B, N), dtype=np.float32)
    for c in range(NCORES):
        cs, br = divmod(c, BR)
        oT = results[c]["outT"].astype(np.float32)
        out[br * MB:(br + 1) * MB, cs * NC_:(cs + 1) * NC_] = \
            oT.reshape(NC_, MB).T
    return np.ascontiguousarray(out)


def kernel(sparse, weight, b, **run_kwargs):
    nc = get_nc()
    in_maps = shard_inputs(sparse, weight, b)
    res = run_bass_kernel_spmd(nc, in_maps, core_ids=list(range(NCORES)),
                               **run_kwargs)
    out = unshard_output(res.results)
    if run_kwargs:
        _CACHE["last_result"] = res
    return out
